# revision 1
# baseline (speedup 1.0000x reference)
"""Trainium2 Bass kernel: out = 2 * cummax_W(cummax_H(x)) for x [16,256,128,128] f32.

Strategy (per core, data-parallel over batch across 8 cores):
  - Each core owns 2 batches -> 512 (b,c) slices of [H=128, W=128].
  - Load G slices per supertile into SBUF as [p=H, f=(g,W)] (one big DMA).
  - W-scan: one segmented cummax via tensor_tensor_scan(op0=add, op1=max)
    with a bias tile that is 0 everywhere and -BIG at each slice's first
    column (resets the running max at slice boundaries).
  - PE-transpose each slice into PSUM ([p=W, f=H]).
  - H-scan: segmented cummax over the transposed data (PSUM -> SBUF).
  - PE-transpose back to natural orientation in PSUM.
  - ACT copies PSUM -> SBUF with x2 scaling (exact for fp32).
  - Store supertile back to DRAM.

All arithmetic is max / x2 / data movement, so the result is bit-exact
vs the fp32 reference.
"""

from contextlib import ExitStack

import numpy as np

import concourse.bass as bass
import concourse.tile as tile
from concourse import bacc, mybir
from concourse.bass_utils import run_bass_kernel_spmd
from concourse.masks import make_identity

N_CORES = 8
B, C, H, W = 16, 256, 128, 128
S = (B // N_CORES) * C  # slices per core
BANK = 512  # fp32 elements per partition in one PSUM bank (4 slices)
NEG = -3.0e38  # effectively -inf for randn-scaled data, finite for safety

F32 = mybir.dt.float32
F32R = mybir.dt.float32r
BF16 = mybir.dt.bfloat16

# Stashed results of the last run (for profiling from test harnesses).
LAST_RESULTS = None


def build_nc(
    n_slices: int = S,
    g: int = 8,
    psum_banks: int = 2,  # PSUM tile width in banks (slices_per_scan = 4*banks)
    f32r_transpose: bool = False,
    warm_every: int = 0,  # issue a tiny bf16 matmul every N transposes (0=off)
    store_engine: str = "scalar",  # second HWDGE ring for stores
    bufs: int = 3,
) -> bass.Bass:
    nc = bacc.Bacc(None, target_bir_lowering=False)
    x = nc.declare_dram_parameter("x", [n_slices, H, W], F32, isOutput=False)
    o = nc.declare_dram_parameter("o", [n_slices, H, W], F32, isOutput=True)

    n_super = n_slices // g
    assert n_super * g == n_slices
    scan_w = psum_banks * BANK  # free width of one H-scan (PSUM)
    spb = scan_w // W  # slices per H-scan
    assert (g * W) % scan_w == 0
    scans_per_super = (g * W) // scan_w

    tdt = F32R if f32r_transpose else F32
    store_eng = getattr(nc, store_engine)

    with ExitStack() as ctx:
        tc = ctx.enter_context(tile.TileContext(nc))
        consts = ctx.enter_context(tc.tile_pool(name="consts", bufs=1))
        ident = consts.tile([128, 128], F32)
        make_identity(nc, ident)
        # Segmented-scan bias: 0 everywhere, NEG at each slice's first column.
        bias = consts.tile([128, g * W], F32)
        nc.vector.memset(bias, 0.0)
        for gi in range(g):
            nc.vector.memset(bias[:, gi * W : gi * W + 1], NEG)
        if warm_every:
            warm_w = consts.tile([128, 2], BF16)
            nc.vector.memset(warm_w, 1.0)

        xpool = ctx.enter_context(tc.tile_pool(name="xt", bufs=bufs))
        apool = ctx.enter_context(tc.tile_pool(name="at", bufs=bufs))
        bpool = ctx.enter_context(tc.tile_pool(name="bt", bufs=bufs))
        opool = ctx.enter_context(tc.tile_pool(name="ot", bufs=bufs))
        pa_pool = ctx.enter_context(tc.tile_pool(name="pa", bufs=2, space="PSUM"))
        pb_pool = ctx.enter_context(tc.tile_pool(name="pb", bufs=2, space="PSUM"))
        if warm_every:
            pw_pool = ctx.enter_context(tc.tile_pool(name="pw", bufs=1, space="PSUM"))

        xv = x.ap().rearrange("(n g) h w -> n g h w", g=g)
        ov = o.ap().rearrange("(n g) h w -> n g h w", g=g)

        n_transposes = 0

        def maybe_warm():
            # A tiny real bf16 matmul counts as PE-busy (transpose-mode does
            # not), keeping the HAM clock gate at full speed.
            nonlocal n_transposes
            n_transposes += 1
            if warm_every and n_transposes % warm_every == 0:
                pw = pw_pool.tile([128, 2], F32)
                nc.tensor.matmul(pw, warm_w, warm_w)

        for t in range(n_super):
            xt = xpool.tile([128, g * W], F32)
            nc.sync.dma_start(
                out=xt[:].rearrange("p (g w) -> p g w", w=W),
                in_=xv[t].rearrange("g h w -> h g w"),
            )
            # cummax along W within each slice (segmented over the g slices)
            at = apool.tile([128, g * W], F32)
            nc.vector.tensor_tensor_scan(
                at[:],
                bias[:],
                xt[:],
                0.0,
                mybir.AluOpType.add,
                mybir.AluOpType.max,
            )
            # Transpose slices into PSUM, then cummax along H (now free dim)
            bt = bpool.tile([128, g * W], F32)
            for hb in range(scans_per_super):
                pa = pa_pool.tile([128, scan_w], F32)
                for j in range(spb):
                    gi = hb * spb + j
                    nc.tensor.transpose(
                        pa[:, j * W : (j + 1) * W].bitcast(tdt),
                        at[:, gi * W : (gi + 1) * W].bitcast(tdt),
                        ident[:].bitcast(tdt),
                    )
                    maybe_warm()
                nc.vector.tensor_tensor_scan(
                    bt[:, hb * scan_w : (hb + 1) * scan_w],
                    bias[:, :scan_w],
                    pa[:],
                    0.0,
                    mybir.AluOpType.add,
                    mybir.AluOpType.max,
                )
            # Transpose back to natural orientation and double via ACT
            ot = opool.tile([128, g * W], F32)
            for hb in range(scans_per_super):
                pb = pb_pool.tile([128, scan_w], F32)
                for j in range(spb):
                    gi = hb * spb + j
                    nc.tensor.transpose(
                        pb[:, j * W : (j + 1) * W].bitcast(tdt),
                        bt[:, gi * W : (gi + 1) * W].bitcast(tdt),
                        ident[:].bitcast(tdt),
                    )
                    maybe_warm()
                nc.scalar.mul(ot[:, hb * scan_w : (hb + 1) * scan_w], pb[:], 2.0)
            store_eng.dma_start(
                out=ov[t].rearrange("g h w -> h g w"),
                in_=ot[:].rearrange("p (g w) -> p g w", w=W),
            )
    nc.finalize()
    return nc


def build_nc_quad(
    n_slices: int = S,
    g: int = 16,  # slices per supertile (multiple of 4)
    bufs: int = 4,
    taper: int = 0,  # number of g//4-sized supertiles at each end
) -> bass.Bass:
    """Quad layout: partition p = s_lo*32 + h_hi (4 slices x 32 h-groups),
    h = h_hi*4 + h_lo. Each DMA descriptor covers 4 h-rows = 2KB contiguous
    DRAM, doubling DMA efficiency vs the natural layout's 512B lines.

    The PE transposes stay [128,128]: chunk (q, h_lo) of the W-scanned tile
    is [p=(s_lo,h_hi), f=w] -> transposed to [p=w, f=(s_lo,h_hi)], written
    strided into PSUM so each quad's H data is linear: free = s_lo*128 + h.
    """
    nc = bacc.Bacc(None, target_bir_lowering=False)
    x = nc.declare_dram_parameter("x", [n_slices, H, W], F32, isOutput=False)
    o = nc.declare_dram_parameter("o", [n_slices, H, W], F32, isOutput=True)

    assert g % 4 == 0
    # Schedule: small supertiles at both ends (faster pipeline fill/drain),
    # full-size in the middle. Entries are (start_slice, n_slices_this).
    gs = g // 4
    chunks = []
    pos = 0
    for _ in range(taper):
        chunks.append((pos, gs))
        pos += gs
    tail_start = n_slices - taper * gs
    while pos < tail_start:
        chunks.append((pos, g))
        pos += g
    for _ in range(taper):
        chunks.append((pos, gs))
        pos += gs
    assert pos == n_slices and all((c % 4 == 0) for _, c in chunks)

    def dram_ap(handle, s0, gc):
        # [p=(s_lo,h_hi):128] [q:nq] [h_lo:4] [w:128], element offset of
        # slice s0; partition stride 512 elems (4 h-rows), quad stride
        # 4 slices.
        return bass.AP(
            tensor=handle,
            offset=s0 * H * W,
            ap=[[512, 128], [4 * H * W, gc // 4], [W, 4], [1, W]],
        )

    with ExitStack() as ctx:
        tc = ctx.enter_context(tile.TileContext(nc))
        consts = ctx.enter_context(tc.tile_pool(name="consts", bufs=1))
        ident = consts.tile([128, 128], F32)
        make_identity(nc, ident)
        bias = consts.tile([128, g * W], F32)
        nc.vector.memset(bias, 0.0)
        for gi in range(g):
            nc.vector.memset(bias[:, gi * W : gi * W + 1], NEG)

        xpool = ctx.enter_context(tc.tile_pool(name="xt", bufs=bufs))
        apool = ctx.enter_context(tc.tile_pool(name="at", bufs=bufs))
        bpool = ctx.enter_context(tc.tile_pool(name="bt", bufs=bufs))
        opool = ctx.enter_context(tc.tile_pool(name="ot", bufs=bufs))
        # pa/pb tiles are 2 banks ([128,1024] = 2 quads); bufs=2 each -> 8 banks
        pa_pool = ctx.enter_context(tc.tile_pool(name="pa", bufs=2, space="PSUM"))
        pb_pool = ctx.enter_context(tc.tile_pool(name="pb", bufs=2, space="PSUM"))

        for s0, gc in chunks:
            nq = gc // 4
            fw = gc * W
            xt = xpool.tile([128, fw], F32, tag="xt")
            nc.sync.dma_start(
                out=xt[:].rearrange("p (q hl w) -> p q hl w", q=nq, hl=4),
                in_=dram_ap(x, s0, gc),
            )
            at = apool.tile([128, fw], F32, tag="at")
            nc.vector.tensor_tensor_scan(
                at[:], bias[:, :fw], xt[:], 0.0, mybir.AluOpType.add, mybir.AluOpType.max
            )
            bt = bpool.tile([128, fw], F32, tag="bt")
            for grp0 in range(0, nq, 2):  # one pa tile = up to 2 quads
                gq = min(2, nq - grp0)
                pw = gq * 512
                pa = pa_pool.tile([128, pw], F32, tag="pa")
                for qs in range(gq):
                    q = grp0 + qs
                    # scatter target: [p=w][s_lo: step 128][h_hi: step 4] + h_lo
                    pav = pa[:].rearrange(
                        "p (qs sl hh f) -> p qs sl hh f", qs=gq, sl=4, hh=32
                    )
                    for hl in range(4):
                        # The 4 strided transposes of one bank form one
                        # accumulation group (disjoint regions, overwrite mode).
                        nc.tensor.matmul(
                            pav[:, qs, :, :, hl],
                            at[:, (q * 4 + hl) * W : (q * 4 + hl + 1) * W],
                            ident[:],
                            start=(hl == 0),
                            stop=(hl == 3),
                            is_transpose=True,
                        )
                nc.vector.tensor_tensor_scan(
                    bt[:, grp0 * 512 : grp0 * 512 + pw],
                    bias[:, :pw],
                    pa[:],
                    0.0,
                    mybir.AluOpType.add,
                    mybir.AluOpType.max,
                )
            ot = opool.tile([128, fw], F32, tag="ot")
            for grp0 in range(0, nq, 2):
                gq = min(2, nq - grp0)
                pw = gq * 512
                pb = pb_pool.tile([128, pw], F32, tag="pb")
                for qs in range(gq):
                    q = grp0 + qs
                    btv = bt[:].rearrange(
                        "p (q sl hh f) -> p q sl hh f", q=nq, sl=4, hh=32
                    )
                    for hl in range(4):
                        nc.tensor.transpose(
                            pb[:, (qs * 4 + hl) * W : (qs * 4 + hl + 1) * W],
                            btv[:, q, :, :, hl],
                            ident[:],
                        )
                nc.scalar.mul(ot[:, grp0 * 512 : grp0 * 512 + pw], pb[:], 2.0)
            nc.gpsimd.dma_start(
                out=dram_ap(o, s0, gc),
                in_=ot[:].rearrange("p (q hl w) -> p q hl w", q=nq, hl=4),
            )
    nc.finalize()
    return nc


def kernel(x: np.ndarray) -> np.ndarray:
    global LAST_RESULTS
    x = np.asarray(x, dtype=np.float32)
    assert x.shape == (B, C, H, W)
    nc = build_nc_quad(S, g=16, bufs=4, taper=4)
    xs = np.ascontiguousarray(x.reshape(N_CORES, S, H, W))
    in_maps = [{"x": xs[i]} for i in range(N_CORES)]
    res = run_bass_kernel_spmd(nc, in_maps, core_ids=list(range(N_CORES)))
    LAST_RESULTS = res
    out = np.stack([res.results[i]["o"] for i in range(N_CORES)])
    return out.reshape(B, C, H, W)



# revision 3
# speedup vs baseline: 1.4437x; 1.4437x over previous
"""Trainium2 Bass kernel: out = 2 * cummax_W(cummax_H(x)) for x [16,256,128,128] f32.

Strategy (per core, data-parallel over batch across 8 cores):
  - Each core owns 2 batches -> 512 (b,c) slices of [H=128, W=128].
  - Quad DRAM layout: partition p = s_lo*32 + h_hi, so each DMA line covers
    4 h-rows = 2KB contiguous DRAM.
  - W-scan: custom DVE op (segmented cummax, 1 elem/cycle, resets at each
    slice's first column via a SUB_DIM_DONE step state).
  - PE-transpose each slice into PSUM ([p=W, f=H]); custom-op H-scan; PE
    transpose back; ACT copies PSUM -> SBUF with x2 scaling; store.

All arithmetic is max / x2 / data movement, so the result is bit-exact
vs the fp32 reference.
"""

import dataclasses
from contextlib import ExitStack

import numpy as np

import concourse.bass as bass
import concourse.dve_ops as dve_ops
import concourse.dve_spec as D
import concourse.tile as tile
from concourse import bacc, mybir
from concourse.bass_utils import run_bass_kernel_spmd
from concourse.dve_uop import DveOpSpec
from concourse.masks import make_identity

N_CORES = 8
B, C, H, W = 16, 256, 128, 128
S = (B // N_CORES) * C  # slices per core

F32 = mybir.dt.float32

LAST_RESULTS = None


# --- custom DVE op: segmented cummax (reset at [P,S,N] page boundaries) ----- #

def _lower_seg_cummax(spec, ver):
    n_lanes, n_stages = D.N_LANES[ver], D.N_STAGES[ver]
    D._validate_body(spec, ver)
    spec2 = D._hoist_stream_invariant_ops(spec)
    scans = D._collect(spec2.body, D.Scan)
    latches = D._collect(spec2.body, D.Latch)
    assert len(scans) == 1 and not latches
    p = D._build_placement(spec2, scans, n_stages, n_lanes)
    states = D._build_state_machine(spec2, scans, latches, p)
    assert len(states) == 2
    seed, steady = states
    d = p.node_stage[scans[0]]
    sg = p.pipeline[d]  # _Stage(MAX, CURR_ALU_OUT, <Src0 route>)
    step_ov = {d: D._Stage(D.AluOp.BYPASS, sg.b)}
    steady2 = dataclasses.replace(
        steady,
        trigger=(D.Trigger.SRC_TENSOR_DONE, D.Trigger.SUB_DIM_DONE, D.Trigger.NONE),
        next=(0, 2, 0),
    )
    step = dataclasses.replace(
        steady,
        overrides=step_ov,
        trigger=(D.Trigger.SRC_TENSOR_DONE, D.Trigger.SUB_DIM_DONE, D.Trigger.COUNT),
        next=(0, 2, 1),
        repeat=1,
    )
    out = [D._assemble(s) for s in (states[0], steady2, step)]
    for u in out:
        u.validate(ver)
    return out


@dataclasses.dataclass(frozen=True)
class _HandDveOp(dve_ops.DveOp):
    def compile(self, ver):
        key = (self.name, ver)
        if (r := dve_ops._COMPILE_CACHE.get(key)) is not None:
            return r
        result = DveOpSpec(
            name=self.name,
            opcode=dve_ops.get_dve_sub_opcode(self.name),
            uops=_lower_seg_cummax(self.spec, ver),
            rd1_en=False,
        )
        dve_ops._COMPILE_CACHE[key] = result
        return result


def _seg_cummax_ref(in0, in1, c0, c1, c2):
    return np.maximum.accumulate(np.asarray(in0, np.float32), axis=-1)


def get_seg_cummax_op():
    for op in dve_ops.OPS:
        if op.name == "SEG_CUMMAX_ANT":
            return op
    spec = D.Spec(
        body=D.scan(D.AluOp.MAX, D.Src0, init=D.MaxNeg),
        reference=_seg_cummax_ref,
    )
    op = _HandDveOp(name="SEG_CUMMAX_ANT", spec=spec, subdim=True, uops_sha={})
    dve_ops.OPS.append(op)
    dve_ops._SUB_OPCODE_FOR_NAME[op.name] = (
        dve_ops._CUSTOM_DVE_ROW_BASE + len(dve_ops.OPS) - 1
    )
    dve_ops.CUSTOM_DVE_SPECS[op.name] = spec
    return op


def seg_cummax(nc, out, in_):
    """out[p,s,:] = cummax(in_[p,s,:]) per page; APs must be [P, S, N]."""
    return nc.vector._custom_dve(get_seg_cummax_op(), out=out, in0=in_)


# --- kernel ----------------------------------------------------------------- #

def build_nc_quad(
    n_slices: int = S,
    g: int = 16,  # slices per supertile (multiple of 4)
    bufs: int = 4,
    taper: int = 0,  # number of g//4-sized supertiles at each end
    psum_quads: int = 2,  # quads per PSUM tile (1 quad = 4 slices = 1 bank)
) -> bass.Bass:
    """Quad layout: partition p = s_lo*32 + h_hi (4 slices x 32 h-groups),
    h = h_hi*4 + h_lo. Each DMA descriptor covers 4 h-rows = 2KB contiguous
    DRAM."""
    nc = bacc.Bacc(None, target_bir_lowering=False)
    x = nc.declare_dram_parameter("x", [n_slices, H, W], F32, isOutput=False)
    o = nc.declare_dram_parameter("o", [n_slices, H, W], F32, isOutput=True)

    assert g % 4 == 0
    gs = g // 4
    chunks = []
    pos = 0
    for _ in range(taper):
        chunks.append((pos, gs))
        pos += gs
    tail_start = n_slices - taper * gs
    while pos < tail_start:
        chunks.append((pos, g))
        pos += g
    for _ in range(taper):
        chunks.append((pos, gs))
        pos += gs
    assert pos == n_slices and all((c % 4 == 0) for _, c in chunks)

    def dram_ap(handle, s0, gc):
        return bass.AP(
            tensor=handle,
            offset=s0 * H * W,
            ap=[[512, 128], [4 * H * W, gc // 4], [W, 4], [1, W]],
        )

    with ExitStack() as ctx:
        tc = ctx.enter_context(tile.TileContext(nc))
        consts = ctx.enter_context(tc.tile_pool(name="consts", bufs=1))
        ident = consts.tile([128, 128], F32)
        make_identity(nc, ident)

        xpool = ctx.enter_context(tc.tile_pool(name="xt", bufs=bufs))
        apool = ctx.enter_context(tc.tile_pool(name="at", bufs=bufs))
        bpool = ctx.enter_context(tc.tile_pool(name="bt", bufs=bufs))
        opool = ctx.enter_context(tc.tile_pool(name="ot", bufs=bufs))
        pa_pool = ctx.enter_context(tc.tile_pool(name="pa", bufs=2, space="PSUM"))
        pb_pool = ctx.enter_context(tc.tile_pool(name="pb", bufs=2, space="PSUM"))

        for s0, gc in chunks:
            nq = gc // 4
            fw = gc * W
            xt = xpool.tile([128, fw], F32, tag="xt")
            nc.sync.dma_start(
                out=xt[:].rearrange("p (q hl w) -> p q hl w", q=nq, hl=4),
                in_=dram_ap(x, s0, gc),
            )
            # W-cummax: each partition's free dim is gc*W with W-length rows
            at = apool.tile([128, fw], F32, tag="at")
            seg_cummax(
                nc,
                at[:].rearrange("p (s n) -> p s n", n=W),
                xt[:].rearrange("p (s n) -> p s n", n=W),
            )
            bt = bpool.tile([128, fw], F32, tag="bt")
            for grp0 in range(0, nq, psum_quads):
                gq = min(psum_quads, nq - grp0)
                pw = gq * 512
                pa = pa_pool.tile([128, pw], F32, tag="pa")
                for qs in range(gq):
                    q = grp0 + qs
                    pav = pa[:].rearrange(
                        "p (qs sl hh f) -> p qs sl hh f", qs=gq, sl=4, hh=32
                    )
                    for hl in range(4):
                        nc.tensor.matmul(
                            pav[:, qs, :, :, hl],
                            at[:, (q * 4 + hl) * W : (q * 4 + hl + 1) * W],
                            ident[:],
                            start=(hl == 0),
                            stop=(hl == 3),
                            is_transpose=True,
                        )
                # H-cummax over the transposed data (segments of 128)
                seg_cummax(
                    nc,
                    bt[:, grp0 * 512 : grp0 * 512 + pw].rearrange(
                        "p (s n) -> p s n", n=128
                    ),
                    pa[:].rearrange("p (s n) -> p s n", n=128),
                )
            ot = opool.tile([128, fw], F32, tag="ot")
            for grp0 in range(0, nq, psum_quads):
                gq = min(psum_quads, nq - grp0)
                pw = gq * 512
                pb = pb_pool.tile([128, pw], F32, tag="pb")
                for qs in range(gq):
                    q = grp0 + qs
                    btv = bt[:].rearrange(
                        "p (q sl hh f) -> p q sl hh f", q=nq, sl=4, hh=32
                    )
                    for hl in range(4):
                        nc.tensor.transpose(
                            pb[:, (qs * 4 + hl) * W : (qs * 4 + hl + 1) * W],
                            btv[:, q, :, :, hl],
                            ident[:],
                        )
                nc.scalar.mul(ot[:, grp0 * 512 : grp0 * 512 + pw], pb[:], 2.0)
            nc.gpsimd.dma_start(
                out=dram_ap(o, s0, gc),
                in_=ot[:].rearrange("p (q hl w) -> p q hl w", q=nq, hl=4),
            )
    nc.finalize()
    return nc


def kernel(x: np.ndarray) -> np.ndarray:
    global LAST_RESULTS
    x = np.asarray(x, dtype=np.float32)
    assert x.shape == (B, C, H, W)
    nc = build_nc_quad(S, g=16, bufs=4, taper=4)
    xs = np.ascontiguousarray(x.reshape(N_CORES, S, H, W))
    in_maps = [{"x": xs[i]} for i in range(N_CORES)]
    res = run_bass_kernel_spmd(nc, in_maps, core_ids=list(range(N_CORES)))
    LAST_RESULTS = res
    out = np.stack([res.results[i]["o"] for i in range(N_CORES)])
    return out.reshape(B, C, H, W)
